# revision 71
# baseline (speedup 1.0000x reference)
"""Trainium2 Bass kernel for nn_CCR_59193239273568 (3-pass spatial attention block).

Strategy (8 NeuronCores, SPMD):
  - Each core owns an 8-image-row band (512 px) of BOTH samples.
  - Phase A: per-band double-conv q/k/v projections (im2col 9-tap fp32r matmuls),
    band outputs in bf16 + their TensorE-transposed form (ones column folded in
    for the softmax row-sum) are AllGathered per sample in bf16.
  - Phase B: per (sample, pass) unit the core computes its 512 query rows:
    S^T chunks [128 keys, 512 queries] as K=32 bf16 matmuls, exp on ScalarE
    straight out of PSUM (scale folded into the activation) to bf16.  ctx is
    accumulated QUERY-MAJOR: for each key chunk, 4 matmuls with the es chunk as
    the stationary operand and V^T|ones [128, 33] as the moving operand, so
    each costs only 33 PE rows instead of 512.  Row-sums land per-partition, so
    normalization is a per-partition reciprocal + scalar multiply (no gpsimd
    broadcast).  Normalized ctx is transposed back to [C, px] with TensorE and
    AllGathered per sample in bf16 (with a zeroed 4th slot used as a zero
    source for image-edge halos).
  - Phase C: each core reads its 14-row ctx window (8 own rows + 3-row halos)
    directly from the gathered buffer via partition-id offset arithmetic (edge
    halos read the zero slot), runs the wr/wg/wb convs in bf16, averages (1/3
    folded into w2's ctx input channels host-side), concat with bf16 x, w2, w3,
    writes its output band.  SAME-padding edge effects are fixed with per-core
    host "bias images" (-1e30 on out-of-image rows, so the conv relu zeroes
    them).
"""

import sys

import numpy as np

sys.path.insert(0, "/opt/trn_rl_repo")

import concourse.bacc as bacc
import concourse.bass as bass
import concourse.mybir as mybir
import concourse.tile as tile
from concourse.bass_utils import run_bass_kernel_spmd

F32 = mybir.dt.float32
F32R = mybir.dt.float32r
BF16 = mybir.dt.bfloat16
AF = mybir.ActivationFunctionType
ALU = mybir.AluOpType

B, CIN, C, H, W = 2, 64, 32, 64, 64
R = 8                 # cores
BR = H // R           # 8 band rows per core per sample
PX = BR * W           # 512 band pixels
N = H * W             # 4096
SCALE = float(C) ** -0.5
NCH = N // 128        # 32 key chunks per sample
GS = 2                # exp group size in chunks (PSUM: 2x2 banks + ctx + tr + convC)

A_SZ = C * PX             # 16384  band in [C, PX] orientation
B_SZ = 128 * 132          # band^T chunks [128, 4, 33]: cols 0:32=V^T, col 32=ones
B_OFF = 3 * A_SZ
CONTRIB1 = 3 * A_SZ + 3 * B_SZ   # per-sample phase-A contribution (bf16 elems)
SZ2 = 4 * A_SZ            # phase-B contribution: 3 ctx passes + 1 zero slot

# phase-A conv weights: conv1 fused across q/k/v (shared input), conv2
# stacked on partitions {0,32,64} so each tensor's lhsT base matches its
# q1pad_all slice
# bf16 conv weight packs (phase C)
CONVS_BF = {"r": ("wr", 32), "g": ("wg", 32), "b": ("wb", 32),
            "2": ("w2", 96), "3": ("w3", 32)}
# bias column in the bias table
BIAS_COL = {"q1": 0, "q2": 1, "k1": 2, "k2": 3, "v1": 4, "v2": 5,
            "r": 6, "g": 7, "b": 8, "2": 9, "3": 10}


def build_program():
    nc = bacc.Bacc("TRN2", target_bir_lowering=False, debug=False, num_devices=R)

    xband_d = nc.declare_dram_parameter("xband", [CIN, B, 12, 66], F32, isOutput=False)
    xbandb_d = nc.declare_dram_parameter("xbandb", [CIN, B, 12, 66], BF16, isOutput=False)
    w1all_d = nc.declare_dram_parameter("p_w1all", [CIN, 9, 96], F32, isOutput=False)
    w2all_d = nc.declare_dram_parameter("p_w2all", [96, 9, C], F32, isOutput=False)
    bmlA1_d = nc.declare_dram_parameter("bmlA1", [2, 96], F32, isOutput=False)
    wbd = {}
    for key, (wname, cin) in CONVS_BF.items():
        wbd[key] = nc.declare_dram_parameter("p_" + wname, [cin, 9, C], BF16, isOutput=False)
    bias_d = nc.declare_dram_parameter("biases", [C, 11], F32, isOutput=False)
    # bias+edge-mask matmul operands: lhsT [2, 12, C] (row0 bias_j, row1 -1e30)
    # and per-core rhs [2, rows, W] (row0 ones, row1 edge mask)
    bmlC_d = nc.declare_dram_parameter("bmlC", [2, 12, C], BF16, isOutput=False)
    bmrA_d = nc.declare_dram_parameter("bmrA", [2, 10, W], F32, isOutput=False)
    bmrC_d = nc.declare_dram_parameter("bmrC", [2, 12, W], BF16, isOutput=False)
    bmrD_d = nc.declare_dram_parameter("bmrD", [2, 10, W], BF16, isOutput=False)
    id32_d = nc.declare_dram_parameter("ident32", [32, 32], BF16, isOutput=False)
    id128_d = nc.declare_dram_parameter("ident128", [128, 128], BF16, isOutput=False)
    out_d = nc.declare_dram_parameter("out", [B, C, BR, W], F32, isOutput=True)

    rg = [list(range(R))]

    with tile.TileContext(nc) as tc:
        with (
            tc.tile_pool(name="const", bufs=1) as constp,
            tc.tile_pool(name="persist", bufs=1) as persistp,
            tc.tile_pool(name="kv", bufs=2) as kvp,
            tc.tile_pool(name="exp", bufs=3) as ep,
            tc.tile_pool(name="small", bufs=2) as smallp,
            tc.tile_pool(name="phc", bufs=1) as phcp,
            tc.tile_pool(name="psum_s", bufs=2, space="PSUM") as psum_s,
            tc.tile_pool(name="psum_ctx", bufs=1, space="PSUM") as psum_ctx,
            tc.tile_pool(name="psum_c", bufs=2, space="PSUM") as psum_c,
            tc.tile_pool(name="dram", bufs=1, space="DRAM") as dramp,
        ):
            pid = nc.sync.partition_id()

            _conv_ps_state = [0]

            def conv_psum(shape):
                _conv_ps_state[0] ^= 1
                if _conv_ps_state[0]:
                    return psum_s.tile(shape, F32, tag="s", name="cps")
                return psum_ctx.tile(shape, F32, tag="ctx", name="cps")

            # ---------------- phase-A-critical constants first ----------------
            xband_sb = constp.tile([CIN, B, 12, 66], F32R, tag="xband")
            nc.sync.dma_start(xband_sb[:], xband_d[:].bitcast(F32R))
            w1all_sb = constp.tile([CIN, 9, 96], F32R, tag="w1all")
            nc.sync.dma_start(w1all_sb[:], w1all_d[:].bitcast(F32R))
            w2all_sb = constp.tile([96, 9, C], F32R, tag="w2all")
            nc.sync.dma_start(w2all_sb[:], w2all_d[:].bitcast(F32R))
            bmlA1_sb = constp.tile([2, 96], F32R, tag="bmlA1")
            nc.sync.dma_start(bmlA1_sb[:], bmlA1_d[:].bitcast(F32R))
            bias_sb = constp.tile([C, 11], F32, tag="bias")
            nc.sync.dma_start(bias_sb[:], bias_d[:])
            bmrA_sb = constp.tile([2, 10, W], F32R, tag="bmrA")
            nc.sync.dma_start(bmrA_sb[:], bmrA_d[:].bitcast(F32R))
            id32_sb = constp.tile([32, 32], BF16, tag="id32")
            nc.sync.dma_start(id32_sb[:], id32_d[:])
            zerobf_sb = constp.tile([C, PX], BF16, tag="zerobf")
            nc.vector.memset(zerobf_sb[:], 0.0)

            # warm the exp table early (overlaps with phase A)
            dummy = constp.tile([1, 16], F32, tag="dummy")
            nc.vector.memset(dummy[:], 0.0)
            nc.scalar.activation(dummy[:], dummy[:], AF.Exp)



            def relu_bias(out_ap, psum_ap, bcol):
                # out = max(psum + bias[bcol], 0)
                nc.vector.tensor_scalar(
                    out_ap, psum_ap, bias_sb[:, bcol:bcol + 1], 0.0,
                    ALU.add, ALU.max,
                )

            def relu0(out_ap, psum_ap):
                # bias + edge-mask already folded into the conv as an extra
                # rank-2 matmul, so the epilogue is a bare relu
                nc.vector.tensor_scalar(out_ap, psum_ap, 0.0, None, ALU.max)

            # ---------------- collective buffers ----------------
            contrib1 = [
                dramp.tile([CONTRIB1], BF16, tag=f"c1_{s}", name=f"contrib1_{s}")
                for s in range(B)
            ]
            gath1 = [
                dramp.tile([R, CONTRIB1], BF16, tag=f"g1_{s}", name=f"gath1_{s}",
                           addr_space="Shared")
                for s in range(B)
            ]
            contrib2 = [
                dramp.tile([3 * A_SZ], BF16, tag=f"c2_{s}", name=f"contrib2_{s}")
                for s in range(B)
            ]
            # per-(sample, pass) gather tiles; row 8 is a locally-zeroed row
            # used as the halo source at the image edges
            gath2 = [
                [
                    dramp.tile([R + 1, A_SZ], BF16, tag=f"g2_{s}_{p}",
                               name=f"gath2_{s}_{p}")
                    for p in range(3)
                ]
                for s in range(B)
            ]
            # ---------------- phase A: q/k/v bands ----------------
            # interleave the three tensors' convs so their PE work pipelines
            # against each other's DVE epilogues
            qband = {}    # (s, t) -> [C, BR, W] bf16 SBUF
            for s in range(B):
                # fused q1/k1/v1 conv: one 96-channel conv over the shared input
                q1a = persistp.tile([96, 10, 66], F32R, tag=f"q1a_{s}")
                nc.vector.memset(q1a[:, :, 0:1].bitcast(F32), 0.0)
                nc.vector.memset(q1a[:, :, 65:66].bitcast(F32), 0.0)
                for j0 in (0, 5):
                    ps = conv_psum([96, 5, W])
                    for tap in range(9):
                        dy, dx = divmod(tap, 3)
                        nc.tensor.matmul(
                            ps[:],
                            w1all_sb[:, tap, :],
                            xband_sb[:, s, j0 + dy:j0 + dy + 5, dx:dx + W],
                            start=(tap == 0), stop=False,
                        )
                    nc.tensor.matmul(
                        ps[:], bmlA1_sb[:], bmrA_sb[:, j0:j0 + 5, :],
                        start=False, stop=True,
                    )
                    relu0(q1a[:, j0:j0 + 5, 1:65], ps[:])

                for t, tn in enumerate(("q", "k", "v")):
                    ps = conv_psum([C, BR, W])
                    for tap in range(9):
                        dy, dx = divmod(tap, 3)
                        nc.tensor.matmul(
                            ps[:],
                            w2all_sb[32 * t:32 * t + 32, tap, :],
                            q1a[32 * t:32 * t + 32, dy:dy + BR, dx:dx + W],
                            start=(tap == 0), stop=(tap == 8),
                        )
                    qb = persistp.tile([C, BR, W], BF16, tag=f"qband_{s}_{t}")
                    relu_bias(qb[:], ps[:], BIAS_COL[tn + "2"])
                    qband[(s, t)] = qb

                    # transposed band chunks [128, 4, 33]: cols 0:32 = band^T,
                    # col 32 = ones (rowsum column for the ctx matmul)
                    vtb = persistp.tile([128, 4, 33], BF16, tag=f"vtb_{s}_{t}")
                    nc.vector.memset(vtb[:, :, 32:33], 1.0)
                    qbf = qb[:].rearrange("c a w -> c (a w)")
                    trp = psum_ctx.tile([128, 4, 32], BF16, tag="tr", name="trp")
                    for ii in range(4):
                        nc.tensor.matmul(
                            trp[:, ii, :], qbf[:, 128 * ii:128 * ii + 128],
                            id32_sb[:], is_transpose=True,
                            start=(ii == 0), stop=(ii == 3),
                        )
                    nc.vector.tensor_copy(vtb[:, :, 0:32], trp[:])

                    nc.sync.dma_start(
                        contrib1[s][t * A_SZ:(t + 1) * A_SZ]
                        .rearrange("(c a w) -> c a w", c=C, w=W),
                        qb[:],
                    )
                    nc.sync.dma_start(
                        contrib1[s][B_OFF + t * B_SZ:B_OFF + (t + 1) * B_SZ]
                        .rearrange("(p a w) -> p a w", p=128, a=4, w=33),
                        vtb[:],
                    )

                nc.gpsimd.collective_compute(
                    "AllGather", ALU.bypass, replica_groups=rg,
                    ins=[contrib1[s].opt()], outs=[gath1[s][:]],
                )

            # needed by phase B's transpose-back — load during the first gather
            id128_sb = constp.tile([128, 128], BF16, tag="id128")
            nc.sync.dma_start(id128_sb[:], id128_d[:])

            # ---------------- phase B: attention units ----------------
            for s in range(B):
                for p in range(3):
                    qt, kt, vt = p, (p + 1) % 3, (p + 2) % 3

                    # per-rank loads so the first S matmuls start after one
                    # small DMA instead of the whole-K transfer
                    ksb = kvp.tile([C, R, PX], BF16, tag="ksb")
                    for rr_ in range(R):
                        nc.sync.dma_start(
                            ksb[:, rr_, :],
                            gath1[s][rr_, kt * A_SZ:(kt + 1) * A_SZ]
                            .rearrange("(c px) -> c px", c=C),
                        )
                    vtsb = kvp.tile([128, R, 4, 33], BF16, tag="vtsb")
                    nc.sync.dma_start(
                        vtsb[:],
                        gath1[s][:, B_OFF + vt * B_SZ:B_OFF + (vt + 1) * B_SZ]
                        .rearrange("g (p a w) -> p g a w", p=128, a=4, w=33),
                    )
                    qrhs = qband[(s, qt)][:].rearrange("c a w -> c (a w)")

                    ctxps = psum_ctx.tile([128, 4 * 33], F32, tag="ctx")
                    ngroups = (NCH + GS - 1) // GS

                    def emit_s_group(g):
                        csz = min(GS, NCH - g * GS)
                        sps = psum_s.tile([128, GS * PX], F32, tag="s", name="sps")
                        for ci in range(csz):
                            i = g * GS + ci
                            rr, ip = divmod(i, 4)
                            nc.tensor.matmul(
                                sps[:, ci * PX:(ci + 1) * PX],
                                ksb[:, rr, 128 * ip:128 * ip + 128],
                                qrhs,
                                start=True, stop=True,
                            )
                        return sps, csz

                    # software pipeline: emit S(g+1) before ctx(g) so the PE
                    # stream never blocks on exp(g) before starting S(g+1)
                    sps, csz = emit_s_group(0)
                    for g in range(ngroups):
                        es = ep.tile([128, GS * PX], BF16, tag="e")
                        nc.scalar.activation(
                            es[:, 0:csz * PX], sps[:, 0:csz * PX], AF.Exp, scale=SCALE
                        )
                        cur_csz = csz
                        if g + 1 < ngroups:
                            sps, csz = emit_s_group(g + 1)
                        for ci in range(cur_csz):
                            i = g * GS + ci
                            rr, ip = divmod(i, 4)
                            for qq in range(4):
                                # start=True zeroes the whole 2KB psum bank, so
                                # only the very first matmul starts; the other
                                # qq slices accumulate onto the zeroed region
                                nc.tensor.matmul(
                                    ctxps[:, qq * 33:(qq + 1) * 33],
                                    es[:, ci * PX + qq * 128:ci * PX + (qq + 1) * 128],
                                    vtsb[:, rr, ip, :],
                                    start=(i == 0 and qq == 0),
                                    stop=(i == NCH - 1 and qq == 3),
                                )

                    # normalization: per-partition softmax denominators live in
                    # column 32 of each 33-column query-quarter group
                    ctxv = ctxps[:].rearrange("p (q t) -> p q t", t=33)
                    recip = smallp.tile([128, 4], F32, tag="recip")
                    nc.vector.reciprocal(recip[:], ctxv[:, :, 32])
                    ctxn = smallp.tile([128, 4, 32], BF16, tag="ctxn")
                    for qq in range(4):
                        nc.vector.tensor_scalar(
                            ctxn[:, qq, :], ctxv[:, qq, 0:32],
                            recip[:, qq:qq + 1], None, ALU.mult,
                        )
                    # transpose back to [C, px]
                    trps = psum_ctx.tile([32, PX], BF16, tag="tr", name="trps")
                    for qq in range(4):
                        nc.tensor.matmul(
                            trps[:, qq * 128:(qq + 1) * 128], ctxn[:, qq, :],
                            id128_sb[:], is_transpose=True,
                            start=(qq == 0), stop=(qq == 3),
                        )
                    ctxT = smallp.tile([C, PX], BF16, tag="ctxT")
                    nc.vector.tensor_copy(ctxT[:], trps[:])
                    nc.sync.dma_start(
                        contrib2[s][p * A_SZ:(p + 1) * A_SZ].rearrange("(c f) -> c f", c=C),
                        ctxT[:],
                    )
                    # gather this pass's ctx right away so phase-C convs for
                    # it can overlap the remaining attention units
                    nc.gpsimd.collective_compute(
                        "AllGather", ALU.bypass, replica_groups=rg,
                        ins=[contrib2[s][p * A_SZ:(p + 1) * A_SZ]],
                        outs=[gath2[s][p][0:R, 0:A_SZ]],
                    )

            # phase-C constants and zero halo rows — emitted after phase B so
            # their DMAs never queue ahead of the first attention unit's loads
            wb_sb = {}
            for key, (wname, cin) in CONVS_BF.items():
                t = constp.tile([cin, 9, C], BF16, tag="wb" + key)
                nc.sync.dma_start(t[:], wbd[key][:])
                wb_sb[key] = t
            bmlC_sb = constp.tile([2, 12, C], BF16, tag="bmlC")
            nc.sync.dma_start(bmlC_sb[:], bmlC_d[:])
            bmrC_sb = constp.tile([2, 12, W], BF16, tag="bmrC")
            nc.sync.dma_start(bmrC_sb[:], bmrC_d[:])
            bmrD_sb = constp.tile([2, 10, W], BF16, tag="bmrD")
            nc.sync.dma_start(bmrD_sb[:], bmrD_d[:])
            xbandb_sb = constp.tile([CIN, B, 12, 66], BF16, tag="xbandb")
            nc.sync.dma_start(xbandb_sb[:], xbandb_d[:])
            for s in range(B):
                for p in range(3):
                    nc.sync.dma_start(
                        gath2[s][p][R, 0:A_SZ].rearrange("(c x) -> c x", c=C),
                        zerobf_sb[:],
                    )

            # ---------------- phase C: output convs ----------------
            # per-core window offsets into the gathered ctx (flat element view);
            # rank 8 of each gather tile is the zero row for image-edge halos
            ASF = A_SZ                    # per-rank stride
            is0 = (8 - pid) // 8          # 1 iff pid == 0
            is7 = (pid + 1) // 8          # 1 iff pid == 7
            r_top = nc.s_assert_le((pid + 7) % 8 + is0, 8)
            r_bot = nc.s_assert_le((pid + 1) % 8 + is7 * 8, 8)
            for s in range(B):
                tmp = {}
                for p, pn in enumerate(("r", "g", "b")):
                    g2f = gath2[s][p][:].rearrange("g z -> (g z)")
                    cpad = phcp.tile([C, 14, 66], BF16, tag="cpad")
                    nc.vector.memset(cpad[:, :, 0:1], 0.0)
                    nc.vector.memset(cpad[:, :, 65:66], 0.0)
                    # own 8 rows
                    nc.sync.dma_start(
                        cpad[:, 3:11, 1:65],
                        g2f[bass.ds(pid * ASF, A_SZ)]
                        .rearrange("(c r w) -> c r w", c=C, w=W),
                    )
                    # top halo: rows 5..7 of rank pid-1 (zero row when pid==0)
                    nc.sync.dma_start(
                        cpad[:, 0:3, 1:65],
                        g2f[bass.ds(r_top * ASF, A_SZ)]
                        .rearrange("(c r w) -> c r w", c=C, w=W)[:, 5:8, :],
                    )
                    # bottom halo: rows 0..2 of rank pid+1 (zero row when pid==7)
                    nc.sync.dma_start(
                        cpad[:, 11:14, 1:65],
                        g2f[bass.ds(r_bot * ASF, A_SZ)]
                        .rearrange("(c r w) -> c r w", c=C, w=W)[:, 0:3, :],
                    )

                    tp = phcp.tile([C, 12, W], F32, tag=f"tmp{p}")
                    for j0 in (0, 6):
                        ps = psum_c.tile([C, 6, W], F32, tag="c", name="cps")
                        for tap in range(9):
                            dy, dx = divmod(tap, 3)
                            nc.tensor.matmul(
                                ps[:],
                                wb_sb[pn][:, tap, :],
                                cpad[:, j0 + dy:j0 + dy + 6, dx:dx + W],
                                start=(tap == 0), stop=False,
                            )
                        nc.tensor.matmul(
                            ps[:], bmlC_sb[:, BIAS_COL[pn], :],
                            bmrC_sb[:, j0:j0 + 6, :], start=False, stop=True,
                        )
                        relu0(tp[:, j0:j0 + 6, :], ps[:])
                    tmp[p] = tp

                xctx = phcp.tile([96, 12, 66], BF16, tag="xctx")
                nc.vector.memset(xctx[:, :, 0:1], 0.0)
                nc.vector.memset(xctx[:, :, 65:66], 0.0)
                nc.vector.tensor_copy(xctx[0:64, :, 1:65], xbandb_sb[:, s, :, 1:65])
                avg = phcp.tile([C, 12, W], F32, tag="avg")
                nc.vector.tensor_add(avg[:], tmp[0][:], tmp[1][:])
                nc.vector.tensor_add(xctx[64:96, :, 1:65], avg[:], tmp[2][:])

                w2pad = phcp.tile([C, 10, 66], BF16, tag="w2pad")
                nc.vector.memset(w2pad[:, :, 0:1], 0.0)
                nc.vector.memset(w2pad[:, :, 65:66], 0.0)
                for j0 in (0, 5):
                    ps = psum_c.tile([C, 5, W], F32, tag="c", name="cps")
                    for tap in range(9):
                        dy, dx = divmod(tap, 3)
                        nc.tensor.matmul(
                            ps[:],
                            wb_sb["2"][:, tap, :],
                            xctx[:, j0 + dy:j0 + dy + 5, dx:dx + W],
                            start=(tap == 0), stop=False,
                        )
                    nc.tensor.matmul(
                        ps[:], bmlC_sb[:, BIAS_COL["2"], :],
                        bmrD_sb[:, j0:j0 + 5, :], start=False, stop=True,
                    )
                    relu0(w2pad[:, j0:j0 + 5, 1:65], ps[:])

                ps = psum_c.tile([C, BR, W], F32, tag="c", name="cps")
                for tap in range(9):
                    dy, dx = divmod(tap, 3)
                    nc.tensor.matmul(
                        ps[:],
                        wb_sb["3"][:, tap, :],
                        w2pad[:, dy:dy + BR, dx:dx + W],
                        start=(tap == 0), stop=(tap == 8),
                    )
                outsb = smallp.tile([C, BR, W], F32, tag="outsb")
                relu_bias(outsb[:], ps[:], BIAS_COL["3"])
                nc.sync.dma_start(out_d[s], outsb[:])

    nc.compile()

    # Re-spell each AllGather's merged flat output AP as the equivalent
    # rank-major 2-D form [[inner, R], [1, inner]] (same contiguous region).
    for f in nc.m.functions:
        for b in f.blocks:
            for i in b.instructions:
                if i.opcode == "CollectiveCompute":
                    lap = i.outs[0]
                    total = 1
                    for _st, cnt in lap.ap:
                        total *= cnt
                    inner = total // R
                    lap.ap = [[inner, R], [1, inner]]
    return nc


def _pack_w(w):
    # [Cout, Cin, 3, 3] -> lhsT pack [Cin, 9, Cout]
    w = np.asarray(w, np.float32)
    return np.ascontiguousarray(w.transpose(1, 2, 3, 0).reshape(w.shape[1], 9, w.shape[0]))


NEG = np.float32(-1e30)


def prep_in_maps(inputs):
    import ml_dtypes

    bf16 = ml_dtypes.bfloat16
    x = np.asarray(inputs["x"], np.float32)
    xp = np.zeros((B, CIN, H + 4, W + 2), np.float32)
    xp[:, :, 2:2 + H, 1:1 + W] = x

    shared = {}
    # fused conv1 pack [CIN, 9, 96] (q|k|v out channels) and partition-stacked
    # conv2 pack [96, 9, C]
    shared["p_w1all"] = np.ascontiguousarray(np.concatenate(
        [_pack_w(inputs["wq1"]), _pack_w(inputs["wk1"]), _pack_w(inputs["wv1"])],
        axis=2,
    ))
    shared["p_w2all"] = np.ascontiguousarray(np.concatenate(
        [_pack_w(inputs["wq2"]), _pack_w(inputs["wk2"]), _pack_w(inputs["wv2"])],
        axis=0,
    ))
    for key, (wname, cin) in CONVS_BF.items():
        w = np.asarray(inputs[wname], np.float32)
        if key == "2":
            w = w.copy()
            w[:, CIN:, :, :] /= 3.0   # fold the ctx 3-way average into w2
        shared["p_" + wname] = _pack_w(w).astype(bf16)
    bnames = ("bq1", "bq2", "bk1", "bk2", "bv1", "bv2", "br", "bg", "bb", "b2", "b3")
    bvals = {bn: np.asarray(inputs[bn], np.float32) for bn in bnames}
    shared["biases"] = np.ascontiguousarray(np.stack([bvals[bn] for bn in bnames], axis=1))
    shared["ident32"] = np.eye(32, dtype=bf16)
    shared["ident128"] = np.eye(128, dtype=bf16)
    # bias+mask lhsT [2, 12, C]: row0 = per-conv bias vector, row1 = -1e30
    bml = np.zeros((2, 12, C), np.float32)
    for j, bn in enumerate(bnames):
        bml[0, j, :] = bvals[bn]
    bml[1, :, :] = NEG
    shared["bmlC"] = bml.astype(bf16)
    bml1 = np.zeros((2, 96), np.float32)
    bml1[0] = np.concatenate([bvals["bq1"], bvals["bk1"], bvals["bv1"]])
    bml1[1] = NEG
    shared["bmlA1"] = bml1

    in_maps = []
    for r in range(R):
        r0 = BR * r
        xband = np.ascontiguousarray(
            xp[:, :, r0:r0 + 12, :].transpose(1, 0, 2, 3)
        )  # [CIN, B, 12, 66]

        # bias+mask rhs [2, rows, W]: row0 = ones (bias), row1 = 1.0 on
        # out-of-image rows (-1e30 after the lhsT, relu'd to the zero SAME
        # padding expects)
        def bmr(rows, top, bot):
            m = np.zeros((2, rows, W), np.float32)
            m[0] = 1.0
            if r == 0:
                m[1, 0:top, :] = 1.0
            if r == R - 1:
                m[1, rows - bot:rows, :] = 1.0
            return m

        bmrA = bmr(10, 1, 1)   # conv1 out rows r0-1 .. r0+8
        bmrC = bmr(12, 2, 2)   # wr/g/b out rows r0-2 .. r0+9
        bmrD = bmr(10, 1, 1)   # w2 out rows r0-1 .. r0+8

        in_maps.append(dict(
            shared, xband=xband, xbandb=xband.astype(bf16),
            bmrA=bmrA, bmrC=bmrC.astype(bf16), bmrD=bmrD.astype(bf16),
        ))
    return in_maps


_CACHE = {}


def get_program():
    if "nc" not in _CACHE:
        _CACHE["nc"] = build_program()
    return _CACHE["nc"]


def kernel(**inputs):
    nc = get_program()
    in_maps = prep_in_maps(inputs)
    res = run_bass_kernel_spmd(nc, in_maps, list(range(R)))
    out = np.zeros((B, C, H, W), np.float32)
    for r in range(R):
        out[:, :, BR * r:BR * (r + 1), :] = res.results[r]["out"]
    return out


# revision 73
# speedup vs baseline: 1.0060x; 1.0060x over previous
"""Trainium2 Bass kernel for nn_CCR_59193239273568 (3-pass spatial attention block).

Strategy (8 NeuronCores, SPMD):
  - Each core owns an 8-image-row band (512 px) of BOTH samples.
  - Phase A: per-band double-conv q/k/v projections (im2col 9-tap fp32r matmuls),
    band outputs in bf16 + their TensorE-transposed form (ones column folded in
    for the softmax row-sum) are AllGathered per sample in bf16.
  - Phase B: per (sample, pass) unit the core computes its 512 query rows:
    S^T chunks [128 keys, 512 queries] as K=32 bf16 matmuls, exp on ScalarE
    straight out of PSUM (scale folded into the activation) to bf16.  ctx is
    accumulated QUERY-MAJOR: for each key chunk, 4 matmuls with the es chunk as
    the stationary operand and V^T|ones [128, 33] as the moving operand, so
    each costs only 33 PE rows instead of 512.  Row-sums land per-partition, so
    normalization is a per-partition reciprocal + scalar multiply (no gpsimd
    broadcast).  Normalized ctx is transposed back to [C, px] with TensorE and
    AllGathered per sample in bf16 (with a zeroed 4th slot used as a zero
    source for image-edge halos).
  - Phase C: each core reads its 14-row ctx window (8 own rows + 3-row halos)
    directly from the gathered buffer via partition-id offset arithmetic (edge
    halos read the zero slot), runs the wr/wg/wb convs in bf16, averages (1/3
    folded into w2's ctx input channels host-side), concat with bf16 x, w2, w3,
    writes its output band.  SAME-padding edge effects are fixed with per-core
    host "bias images" (-1e30 on out-of-image rows, so the conv relu zeroes
    them).
"""

import sys

import numpy as np

sys.path.insert(0, "/opt/trn_rl_repo")

import concourse.bacc as bacc
import concourse.bass as bass
import concourse.mybir as mybir
import concourse.tile as tile
from concourse.bass_utils import run_bass_kernel_spmd

F32 = mybir.dt.float32
F32R = mybir.dt.float32r
BF16 = mybir.dt.bfloat16
AF = mybir.ActivationFunctionType
ALU = mybir.AluOpType

B, CIN, C, H, W = 2, 64, 32, 64, 64
R = 8                 # cores
BR = H // R           # 8 band rows per core per sample
PX = BR * W           # 512 band pixels
N = H * W             # 4096
SCALE = float(C) ** -0.5
NCH = N // 128        # 32 key chunks per sample
GS = 2                # exp group size in chunks (PSUM: 2x2 banks + ctx + tr + convC)

A_SZ = C * PX             # 16384  band in [C, PX] orientation
B_SZ = 128 * 132          # band^T chunks [128, 4, 33]: cols 0:32=V^T, col 32=ones
B_OFF = 3 * A_SZ
CONTRIB1 = 3 * A_SZ + 3 * B_SZ   # per-sample phase-A contribution (bf16 elems)
SZ2 = 4 * A_SZ            # phase-B contribution: 3 ctx passes + 1 zero slot

# phase-A conv weights: conv1 fused across q/k/v (shared input), conv2
# stacked on partitions {0,32,64} so each tensor's lhsT base matches its
# q1pad_all slice
# bf16 conv weight packs (phase C)
CONVS_BF = {"r": ("wr", 32), "g": ("wg", 32), "b": ("wb", 32),
            "2": ("w2", 96), "3": ("w3", 32)}
# bias column in the bias table
BIAS_COL = {"q1": 0, "q2": 1, "k1": 2, "k2": 3, "v1": 4, "v2": 5,
            "r": 6, "g": 7, "b": 8, "2": 9, "3": 10}


def build_program():
    nc = bacc.Bacc("TRN2", target_bir_lowering=False, debug=False, num_devices=R)

    xband_d = nc.declare_dram_parameter("xband", [CIN, B, 12, 66], F32, isOutput=False)
    xbandb_d = nc.declare_dram_parameter("xbandb", [CIN, B, 12, 66], BF16, isOutput=False)
    w1all_d = nc.declare_dram_parameter("p_w1all", [CIN, 9, 96], F32, isOutput=False)
    w2all_d = nc.declare_dram_parameter("p_w2all", [96, 9, C], F32, isOutput=False)
    bmlA1_d = nc.declare_dram_parameter("bmlA1", [2, 96], F32, isOutput=False)
    wbd = {}
    for key, (wname, cin) in CONVS_BF.items():
        wbd[key] = nc.declare_dram_parameter("p_" + wname, [cin, 9, C], BF16, isOutput=False)
    bias_d = nc.declare_dram_parameter("biases", [C, 11], F32, isOutput=False)
    # bias+edge-mask matmul operands: lhsT [2, 12, C] (row0 bias_j, row1 -1e30)
    # and per-core rhs [2, rows, W] (row0 ones, row1 edge mask)
    bmlC_d = nc.declare_dram_parameter("bmlC", [2, 12, C], BF16, isOutput=False)
    bmrA_d = nc.declare_dram_parameter("bmrA", [2, 10, W], F32, isOutput=False)
    bmrC_d = nc.declare_dram_parameter("bmrC", [2, 12, W], BF16, isOutput=False)
    bmrD_d = nc.declare_dram_parameter("bmrD", [2, 10, W], BF16, isOutput=False)
    id32_d = nc.declare_dram_parameter("ident32", [32, 32], BF16, isOutput=False)
    id128_d = nc.declare_dram_parameter("ident128", [128, 128], BF16, isOutput=False)
    out_d = nc.declare_dram_parameter("out", [B, C, BR, W], F32, isOutput=True)

    rg = [list(range(R))]

    with tile.TileContext(nc) as tc:
        with (
            tc.tile_pool(name="const", bufs=1) as constp,
            tc.tile_pool(name="persist", bufs=1) as persistp,
            tc.tile_pool(name="kv", bufs=2) as kvp,
            tc.tile_pool(name="exp", bufs=3) as ep,
            tc.tile_pool(name="small", bufs=2) as smallp,
            tc.tile_pool(name="phc", bufs=1) as phcp,
            tc.tile_pool(name="psum_s", bufs=2, space="PSUM") as psum_s,
            tc.tile_pool(name="psum_ctx", bufs=1, space="PSUM") as psum_ctx,
            tc.tile_pool(name="psum_c", bufs=2, space="PSUM") as psum_c,
            tc.tile_pool(name="dram", bufs=1, space="DRAM") as dramp,
        ):
            pid = nc.sync.partition_id()

            _conv_ps_state = [0]

            def conv_psum(shape):
                _conv_ps_state[0] ^= 1
                if _conv_ps_state[0]:
                    return psum_s.tile(shape, F32, tag="s", name="cps")
                return psum_ctx.tile(shape, F32, tag="ctx", name="cps")

            # ---------------- phase-A-critical constants first ----------------
            xband_sb = constp.tile([CIN, B, 12, 66], F32R, tag="xband")
            nc.sync.dma_start(xband_sb[:], xband_d[:].bitcast(F32R))
            w1all_sb = constp.tile([CIN, 9, 96], F32R, tag="w1all")
            nc.sync.dma_start(w1all_sb[:], w1all_d[:].bitcast(F32R))
            w2all_sb = constp.tile([96, 9, C], F32R, tag="w2all")
            nc.sync.dma_start(w2all_sb[:], w2all_d[:].bitcast(F32R))
            bmlA1_sb = constp.tile([2, 96], F32R, tag="bmlA1")
            nc.sync.dma_start(bmlA1_sb[:], bmlA1_d[:].bitcast(F32R))
            bias_sb = constp.tile([C, 11], F32, tag="bias")
            nc.sync.dma_start(bias_sb[:], bias_d[:])
            bmrA_sb = constp.tile([2, 10, W], F32R, tag="bmrA")
            nc.sync.dma_start(bmrA_sb[:], bmrA_d[:].bitcast(F32R))
            id32_sb = constp.tile([32, 32], BF16, tag="id32")
            nc.sync.dma_start(id32_sb[:], id32_d[:])
            zerobf_sb = constp.tile([C, PX], BF16, tag="zerobf")
            nc.vector.memset(zerobf_sb[:], 0.0)

            # warm the exp table early (overlaps with phase A)
            dummy = constp.tile([1, 16], F32, tag="dummy")
            nc.vector.memset(dummy[:], 0.0)
            nc.scalar.activation(dummy[:], dummy[:], AF.Exp)



            def relu_bias(out_ap, psum_ap, bcol):
                # out = max(psum + bias[bcol], 0)
                nc.vector.tensor_scalar(
                    out_ap, psum_ap, bias_sb[:, bcol:bcol + 1], 0.0,
                    ALU.add, ALU.max,
                )

            def relu0(out_ap, psum_ap):
                # bias + edge-mask already folded into the conv as an extra
                # rank-2 matmul, so the epilogue is a bare relu
                nc.vector.tensor_scalar(out_ap, psum_ap, 0.0, None, ALU.max)

            # ---------------- collective buffers ----------------
            contrib1 = [
                dramp.tile([CONTRIB1], BF16, tag=f"c1_{s}", name=f"contrib1_{s}")
                for s in range(B)
            ]
            gath1 = [
                dramp.tile([R, CONTRIB1], BF16, tag=f"g1_{s}", name=f"gath1_{s}",
                           addr_space="Shared")
                for s in range(B)
            ]
            contrib2 = [
                dramp.tile([3 * A_SZ], BF16, tag=f"c2_{s}", name=f"contrib2_{s}")
                for s in range(B)
            ]
            # per-(sample, pass) gather tiles; row 8 is a locally-zeroed row
            # used as the halo source at the image edges
            gath2 = [
                [
                    dramp.tile([R + 1, A_SZ], BF16, tag=f"g2_{s}_{p}",
                               name=f"gath2_{s}_{p}")
                    for p in range(3)
                ]
                for s in range(B)
            ]
            # ---------------- phase A: q/k/v bands ----------------
            # interleave the three tensors' convs so their PE work pipelines
            # against each other's DVE epilogues
            qband = {}    # (s, t) -> [C, BR, W] bf16 SBUF
            for s in range(B):
                # fused q1/k1/v1 conv: one 96-channel conv over the shared input
                q1a = persistp.tile([96, 10, 66], F32R, tag=f"q1a_{s}")
                nc.vector.memset(q1a[:, :, 0:1].bitcast(F32), 0.0)
                nc.vector.memset(q1a[:, :, 65:66].bitcast(F32), 0.0)
                for j0 in (0, 5):
                    ps = conv_psum([96, 5, W])
                    for tap in range(9):
                        dy, dx = divmod(tap, 3)
                        nc.tensor.matmul(
                            ps[:],
                            w1all_sb[:, tap, :],
                            xband_sb[:, s, j0 + dy:j0 + dy + 5, dx:dx + W],
                            start=(tap == 0), stop=False,
                        )
                    nc.tensor.matmul(
                        ps[:], bmlA1_sb[:], bmrA_sb[:, j0:j0 + 5, :],
                        start=False, stop=True,
                    )
                    relu0(q1a[:, j0:j0 + 5, 1:65], ps[:])

                for t, tn in enumerate(("q", "k", "v")):
                    ps = conv_psum([C, BR, W])
                    for tap in range(9):
                        dy, dx = divmod(tap, 3)
                        nc.tensor.matmul(
                            ps[:],
                            w2all_sb[32 * t:32 * t + 32, tap, :],
                            q1a[32 * t:32 * t + 32, dy:dy + BR, dx:dx + W],
                            start=(tap == 0), stop=(tap == 8),
                        )
                    qb = persistp.tile([C, BR, W], BF16, tag=f"qband_{s}_{t}")
                    relu_bias(qb[:], ps[:], BIAS_COL[tn + "2"])
                    qband[(s, t)] = qb

                    # transposed band chunks [128, 4, 33]: cols 0:32 = band^T,
                    # col 32 = ones (rowsum column for the ctx matmul)
                    vtb = persistp.tile([128, 4, 33], BF16, tag=f"vtb_{s}_{t}")
                    nc.vector.memset(vtb[:, :, 32:33], 1.0)
                    qbf = qb[:].rearrange("c a w -> c (a w)")
                    trp = psum_ctx.tile([128, 4, 32], BF16, tag="tr", name="trp")
                    for ii in range(4):
                        nc.tensor.matmul(
                            trp[:, ii, :], qbf[:, 128 * ii:128 * ii + 128],
                            id32_sb[:], is_transpose=True,
                            start=(ii == 0), stop=(ii == 3),
                        )
                    nc.vector.tensor_copy(vtb[:, :, 0:32], trp[:])

                    nc.sync.dma_start(
                        contrib1[s][t * A_SZ:(t + 1) * A_SZ]
                        .rearrange("(c a w) -> c a w", c=C, w=W),
                        qb[:],
                    )
                    nc.sync.dma_start(
                        contrib1[s][B_OFF + t * B_SZ:B_OFF + (t + 1) * B_SZ]
                        .rearrange("(p a w) -> p a w", p=128, a=4, w=33),
                        vtb[:],
                    )

                nc.gpsimd.collective_compute(
                    "AllGather", ALU.bypass, replica_groups=rg,
                    ins=[contrib1[s].opt()], outs=[gath1[s][:]],
                )

            # needed by phase B's transpose-back — load during the first gather
            id128_sb = constp.tile([128, 128], BF16, tag="id128")
            nc.sync.dma_start(id128_sb[:], id128_d[:])

            # ---------------- phase B: attention units ----------------
            for s in range(B):
                for p in range(3):
                    qt, kt, vt = p, (p + 1) % 3, (p + 2) % 3

                    # per-rank loads so the first S matmuls start after one
                    # small DMA instead of the whole-K transfer
                    ksb = kvp.tile([C, R, PX], BF16, tag="ksb")
                    for rr_ in range(R):
                        nc.sync.dma_start(
                            ksb[:, rr_, :],
                            gath1[s][rr_, kt * A_SZ:(kt + 1) * A_SZ]
                            .rearrange("(c px) -> c px", c=C),
                        )
                    vtsb = kvp.tile([128, R, 4, 33], BF16, tag="vtsb")
                    nc.sync.dma_start(
                        vtsb[:],
                        gath1[s][:, B_OFF + vt * B_SZ:B_OFF + (vt + 1) * B_SZ]
                        .rearrange("g (p a w) -> p g a w", p=128, a=4, w=33),
                    )
                    qrhs = qband[(s, qt)][:].rearrange("c a w -> c (a w)")

                    ctxps = psum_ctx.tile([128, 4 * 33], F32, tag="ctx")
                    ngroups = (NCH + GS - 1) // GS

                    def emit_s_group(g):
                        csz = min(GS, NCH - g * GS)
                        sps = psum_s.tile([128, GS * PX], F32, tag="s", name="sps")
                        for ci in range(csz):
                            i = g * GS + ci
                            rr, ip = divmod(i, 4)
                            nc.tensor.matmul(
                                sps[:, ci * PX:(ci + 1) * PX],
                                ksb[:, rr, 128 * ip:128 * ip + 128],
                                qrhs,
                                start=True, stop=True,
                            )
                        return sps, csz

                    # software pipeline: emit S(g+1) before ctx(g) so the PE
                    # stream never blocks on exp(g) before starting S(g+1)
                    sps, csz = emit_s_group(0)
                    for g in range(ngroups):
                        es = ep.tile([128, GS * PX], BF16, tag="e")
                        nc.scalar.activation(
                            es[:, 0:csz * PX], sps[:, 0:csz * PX], AF.Exp, scale=SCALE
                        )
                        cur_csz = csz
                        if g + 1 < ngroups:
                            sps, csz = emit_s_group(g + 1)
                        for ci in range(cur_csz):
                            i = g * GS + ci
                            rr, ip = divmod(i, 4)
                            for qq in range(4):
                                # start=True zeroes the whole 2KB psum bank, so
                                # only the very first matmul starts; the other
                                # qq slices accumulate onto the zeroed region
                                nc.tensor.matmul(
                                    ctxps[:, qq * 33:(qq + 1) * 33],
                                    es[:, ci * PX + qq * 128:ci * PX + (qq + 1) * 128],
                                    vtsb[:, rr, ip, :],
                                    start=(i == 0 and qq == 0),
                                    stop=(i == NCH - 1 and qq == 3),
                                )

                    # normalization: per-partition softmax denominators live in
                    # column 32 of each 33-column query-quarter group
                    ctxv = ctxps[:].rearrange("p (q t) -> p q t", t=33)
                    recip = smallp.tile([128, 4], F32, tag="recip")
                    nc.vector.reciprocal(recip[:], ctxv[:, :, 32])
                    ctxn = smallp.tile([128, 4, 32], BF16, tag="ctxn")
                    for qq in range(4):
                        nc.vector.tensor_scalar(
                            ctxn[:, qq, :], ctxv[:, qq, 0:32],
                            recip[:, qq:qq + 1], None, ALU.mult,
                        )
                    # transpose back to [C, px]
                    trps = psum_ctx.tile([32, PX], BF16, tag="tr", name="trps")
                    for qq in range(4):
                        nc.tensor.matmul(
                            trps[:, qq * 128:(qq + 1) * 128], ctxn[:, qq, :],
                            id128_sb[:], is_transpose=True,
                            start=(qq == 0), stop=(qq == 3),
                        )
                    ctxT = smallp.tile([C, PX], BF16, tag="ctxT")
                    nc.vector.tensor_copy(ctxT[:], trps[:])
                    if (s, p) == (B - 1, 2):
                        last_ctxT = ctxT
                    nc.sync.dma_start(
                        contrib2[s][p * A_SZ:(p + 1) * A_SZ].rearrange("(c f) -> c f", c=C),
                        ctxT[:],
                    )
                    # gather this pass's ctx right away so phase-C convs for
                    # it can overlap the remaining attention units
                    nc.gpsimd.collective_compute(
                        "AllGather", ALU.bypass, replica_groups=rg,
                        ins=[contrib2[s][p * A_SZ:(p + 1) * A_SZ]],
                        outs=[gath2[s][p][0:R, 0:A_SZ]],
                    )

            # phase-C constants and zero halo rows — emitted after phase B so
            # their DMAs never queue ahead of the first attention unit's loads
            wb_sb = {}
            for key, (wname, cin) in CONVS_BF.items():
                t = constp.tile([cin, 9, C], BF16, tag="wb" + key)
                nc.sync.dma_start(t[:], wbd[key][:])
                wb_sb[key] = t
            bmlC_sb = constp.tile([2, 12, C], BF16, tag="bmlC")
            nc.sync.dma_start(bmlC_sb[:], bmlC_d[:])
            bmrC_sb = constp.tile([2, 12, W], BF16, tag="bmrC")
            nc.sync.dma_start(bmrC_sb[:], bmrC_d[:])
            bmrD_sb = constp.tile([2, 10, W], BF16, tag="bmrD")
            nc.sync.dma_start(bmrD_sb[:], bmrD_d[:])
            xbandb_sb = constp.tile([CIN, B, 12, 66], BF16, tag="xbandb")
            nc.sync.dma_start(xbandb_sb[:], xbandb_d[:])
            for s in range(B):
                for p in range(3):
                    nc.sync.dma_start(
                        gath2[s][p][R, 0:A_SZ].rearrange("(c x) -> c x", c=C),
                        zerobf_sb[:],
                    )

            # ---------------- phase C: output convs ----------------
            # per-core window offsets into the gathered ctx (flat element view);
            # rank 8 of each gather tile is the zero row for image-edge halos
            ASF = A_SZ                    # per-rank stride
            is0 = (8 - pid) // 8          # 1 iff pid == 0
            is7 = (pid + 1) // 8          # 1 iff pid == 7
            r_top = nc.s_assert_le((pid + 7) % 8 + is0, 8)
            r_bot = nc.s_assert_le((pid + 1) % 8 + is7 * 8, 8)
            for s in range(B):
                tmp = {}
                for p, pn in enumerate(("r", "g", "b")):
                    g2f = gath2[s][p][:].rearrange("g z -> (g z)")
                    cpad = phcp.tile([C, 14, 66], BF16, tag="cpad")
                    nc.vector.memset(cpad[:, :, 0:1], 0.0)
                    nc.vector.memset(cpad[:, :, 65:66], 0.0)
                    # own 8 rows
                    nc.sync.dma_start(
                        cpad[:, 3:11, 1:65],
                        g2f[bass.ds(pid * ASF, A_SZ)]
                        .rearrange("(c r w) -> c r w", c=C, w=W),
                    )
                    # top halo: rows 5..7 of rank pid-1 (zero row when pid==0)
                    nc.sync.dma_start(
                        cpad[:, 0:3, 1:65],
                        g2f[bass.ds(r_top * ASF, A_SZ)]
                        .rearrange("(c r w) -> c r w", c=C, w=W)[:, 5:8, :],
                    )
                    # bottom halo: rows 0..2 of rank pid+1 (zero row when pid==7)
                    nc.sync.dma_start(
                        cpad[:, 11:14, 1:65],
                        g2f[bass.ds(r_bot * ASF, A_SZ)]
                        .rearrange("(c r w) -> c r w", c=C, w=W)[:, 0:3, :],
                    )

                    tp = phcp.tile([C, 12, W], F32, tag=f"tmp{p}")
                    if (s, p) == (B - 1, 2):
                        # the whole kernel's tail gates on this conv: pre-run
                        # the interior rows (own-band-only inputs) from the
                        # local ctx band before the gather lands
                        lpad = phcp.tile([C, 8, 66], BF16, tag="lpad")
                        nc.vector.memset(lpad[:, :, 0:1], 0.0)
                        nc.vector.memset(lpad[:, :, 65:66], 0.0)
                        nc.vector.tensor_copy(
                            lpad[:, :, 1:65],
                            last_ctxT[:].rearrange("c (r w) -> c r w", w=W),
                        )
                        ps = psum_c.tile([C, 5, W], F32, tag="c", name="cps")
                        for tap in range(9):
                            dy, dx = divmod(tap, 3)
                            nc.tensor.matmul(
                                ps[:],
                                wb_sb[pn][:, tap, :],
                                lpad[:, dy:dy + 5, dx:dx + W],
                                start=(tap == 0), stop=False,
                            )
                        nc.tensor.matmul(
                            ps[:], bmlC_sb[:, BIAS_COL[pn], :],
                            bmrC_sb[:, 3:8, :], start=False, stop=True,
                        )
                        relu0(tp[:, 3:8, :], ps[:])
                        # post-gather edge rows: out 0..2 (cpad 0..4) and
                        # out 8..11 (cpad 8..13)
                        for o0, rows in ((0, 3), (8, 4)):
                            ps = psum_c.tile([C, rows, W], F32, tag="c", name="cps")
                            for tap in range(9):
                                dy, dx = divmod(tap, 3)
                                nc.tensor.matmul(
                                    ps[:],
                                    wb_sb[pn][:, tap, :],
                                    cpad[:, o0 + dy:o0 + dy + rows, dx:dx + W],
                                    start=(tap == 0), stop=False,
                                )
                            nc.tensor.matmul(
                                ps[:], bmlC_sb[:, BIAS_COL[pn], :],
                                bmrC_sb[:, o0:o0 + rows, :], start=False, stop=True,
                            )
                            relu0(tp[:, o0:o0 + rows, :], ps[:])
                    else:
                        for j0 in (0, 6):
                            ps = psum_c.tile([C, 6, W], F32, tag="c", name="cps")
                            for tap in range(9):
                                dy, dx = divmod(tap, 3)
                                nc.tensor.matmul(
                                    ps[:],
                                    wb_sb[pn][:, tap, :],
                                    cpad[:, j0 + dy:j0 + dy + 6, dx:dx + W],
                                    start=(tap == 0), stop=False,
                                )
                            nc.tensor.matmul(
                                ps[:], bmlC_sb[:, BIAS_COL[pn], :],
                                bmrC_sb[:, j0:j0 + 6, :], start=False, stop=True,
                            )
                            relu0(tp[:, j0:j0 + 6, :], ps[:])
                    tmp[p] = tp

                xctx = phcp.tile([96, 12, 66], BF16, tag="xctx")
                nc.vector.memset(xctx[:, :, 0:1], 0.0)
                nc.vector.memset(xctx[:, :, 65:66], 0.0)
                nc.vector.tensor_copy(xctx[0:64, :, 1:65], xbandb_sb[:, s, :, 1:65])
                avg = phcp.tile([C, 12, W], F32, tag="avg")
                nc.vector.tensor_add(avg[:], tmp[0][:], tmp[1][:])
                nc.vector.tensor_add(xctx[64:96, :, 1:65], avg[:], tmp[2][:])

                w2pad = phcp.tile([C, 10, 66], BF16, tag="w2pad")
                nc.vector.memset(w2pad[:, :, 0:1], 0.0)
                nc.vector.memset(w2pad[:, :, 65:66], 0.0)
                for j0 in (0, 5):
                    ps = psum_c.tile([C, 5, W], F32, tag="c", name="cps")
                    for tap in range(9):
                        dy, dx = divmod(tap, 3)
                        nc.tensor.matmul(
                            ps[:],
                            wb_sb["2"][:, tap, :],
                            xctx[:, j0 + dy:j0 + dy + 5, dx:dx + W],
                            start=(tap == 0), stop=False,
                        )
                    nc.tensor.matmul(
                        ps[:], bmlC_sb[:, BIAS_COL["2"], :],
                        bmrD_sb[:, j0:j0 + 5, :], start=False, stop=True,
                    )
                    relu0(w2pad[:, j0:j0 + 5, 1:65], ps[:])

                ps = psum_c.tile([C, BR, W], F32, tag="c", name="cps")
                for tap in range(9):
                    dy, dx = divmod(tap, 3)
                    nc.tensor.matmul(
                        ps[:],
                        wb_sb["3"][:, tap, :],
                        w2pad[:, dy:dy + BR, dx:dx + W],
                        start=(tap == 0), stop=(tap == 8),
                    )
                outsb = smallp.tile([C, BR, W], F32, tag="outsb")
                relu_bias(outsb[:], ps[:], BIAS_COL["3"])
                nc.sync.dma_start(out_d[s], outsb[:])

    nc.compile()

    # Re-spell each AllGather's merged flat output AP as the equivalent
    # rank-major 2-D form [[inner, R], [1, inner]] (same contiguous region).
    for f in nc.m.functions:
        for b in f.blocks:
            for i in b.instructions:
                if i.opcode == "CollectiveCompute":
                    lap = i.outs[0]
                    total = 1
                    for _st, cnt in lap.ap:
                        total *= cnt
                    inner = total // R
                    lap.ap = [[inner, R], [1, inner]]
    return nc


def _pack_w(w):
    # [Cout, Cin, 3, 3] -> lhsT pack [Cin, 9, Cout]
    w = np.asarray(w, np.float32)
    return np.ascontiguousarray(w.transpose(1, 2, 3, 0).reshape(w.shape[1], 9, w.shape[0]))


NEG = np.float32(-1e30)


def prep_in_maps(inputs):
    import ml_dtypes

    bf16 = ml_dtypes.bfloat16
    x = np.asarray(inputs["x"], np.float32)
    xp = np.zeros((B, CIN, H + 4, W + 2), np.float32)
    xp[:, :, 2:2 + H, 1:1 + W] = x

    shared = {}
    # fused conv1 pack [CIN, 9, 96] (q|k|v out channels) and partition-stacked
    # conv2 pack [96, 9, C]
    shared["p_w1all"] = np.ascontiguousarray(np.concatenate(
        [_pack_w(inputs["wq1"]), _pack_w(inputs["wk1"]), _pack_w(inputs["wv1"])],
        axis=2,
    ))
    shared["p_w2all"] = np.ascontiguousarray(np.concatenate(
        [_pack_w(inputs["wq2"]), _pack_w(inputs["wk2"]), _pack_w(inputs["wv2"])],
        axis=0,
    ))
    for key, (wname, cin) in CONVS_BF.items():
        w = np.asarray(inputs[wname], np.float32)
        if key == "2":
            w = w.copy()
            w[:, CIN:, :, :] /= 3.0   # fold the ctx 3-way average into w2
        shared["p_" + wname] = _pack_w(w).astype(bf16)
    bnames = ("bq1", "bq2", "bk1", "bk2", "bv1", "bv2", "br", "bg", "bb", "b2", "b3")
    bvals = {bn: np.asarray(inputs[bn], np.float32) for bn in bnames}
    shared["biases"] = np.ascontiguousarray(np.stack([bvals[bn] for bn in bnames], axis=1))
    shared["ident32"] = np.eye(32, dtype=bf16)
    shared["ident128"] = np.eye(128, dtype=bf16)
    # bias+mask lhsT [2, 12, C]: row0 = per-conv bias vector, row1 = -1e30
    bml = np.zeros((2, 12, C), np.float32)
    for j, bn in enumerate(bnames):
        bml[0, j, :] = bvals[bn]
    bml[1, :, :] = NEG
    shared["bmlC"] = bml.astype(bf16)
    bml1 = np.zeros((2, 96), np.float32)
    bml1[0] = np.concatenate([bvals["bq1"], bvals["bk1"], bvals["bv1"]])
    bml1[1] = NEG
    shared["bmlA1"] = bml1

    in_maps = []
    for r in range(R):
        r0 = BR * r
        xband = np.ascontiguousarray(
            xp[:, :, r0:r0 + 12, :].transpose(1, 0, 2, 3)
        )  # [CIN, B, 12, 66]

        # bias+mask rhs [2, rows, W]: row0 = ones (bias), row1 = 1.0 on
        # out-of-image rows (-1e30 after the lhsT, relu'd to the zero SAME
        # padding expects)
        def bmr(rows, top, bot):
            m = np.zeros((2, rows, W), np.float32)
            m[0] = 1.0
            if r == 0:
                m[1, 0:top, :] = 1.0
            if r == R - 1:
                m[1, rows - bot:rows, :] = 1.0
            return m

        bmrA = bmr(10, 1, 1)   # conv1 out rows r0-1 .. r0+8
        bmrC = bmr(12, 2, 2)   # wr/g/b out rows r0-2 .. r0+9
        bmrD = bmr(10, 1, 1)   # w2 out rows r0-1 .. r0+8

        in_maps.append(dict(
            shared, xband=xband, xbandb=xband.astype(bf16),
            bmrA=bmrA, bmrC=bmrC.astype(bf16), bmrD=bmrD.astype(bf16),
        ))
    return in_maps


_CACHE = {}


def get_program():
    if "nc" not in _CACHE:
        _CACHE["nc"] = build_program()
    return _CACHE["nc"]


def kernel(**inputs):
    nc = get_program()
    in_maps = prep_in_maps(inputs)
    res = run_bass_kernel_spmd(nc, in_maps, list(range(R)))
    out = np.zeros((B, C, H, W), np.float32)
    for r in range(R):
        out[:, :, BR * r:BR * (r + 1), :] = res.results[r]["out"]
    return out
